# revision 24
# baseline (speedup 1.0000x reference)
"""Trainium2 Bass kernel for nn_ActivationFilter:
y = bicubic_down2x( leaky_relu( bicubic_up2x(x) ) ), x: (8, 128, 128, 128) f32 NHWC.

Since jax.image.resize is a separable linear map, per (batch, channel):
    y = D @ leaky_relu(U @ X @ U^T) @ D^T
with U (256x128) the bicubic 2x-upsample matrix and D (128x256) the
antialiased bicubic downsample matrix.

Sharding: batch-per-core (8 batches over 8 NeuronCores), no collectives.

Per-core algorithm (per channel c, all on TensorE, no transposes):
  Ph1: Z1t[w, h2]  = sum_h  x[h, w, c] * Ut[h, h2]       (lhsT = X_c, f32r)
  Ph2: z[h2, w2]   = sum_w  Z1t[w, h2] * Ut[w, w2]       (lhsT = Z1t half, bf16)
       zs = leaky_relu(z)  (fused into PSUM evacuation on ScalarE)
  Ph3: y3[w2, h3]  = sum_h2 zs[h2, w2] * Dt[h2, h3]      (lhsT = zs slice, bf16)
  Ph4: y[w3, h3]   = sum_w2 Dt[w2, w3]^T-form @ y3       (lhsT = Dw, bf16)
Each phase's output partition dim is the next phase's contraction dim,
so no transposes are ever needed.
"""

import sys
import os

if "/opt/trn_rl_repo" not in sys.path:
    sys.path.insert(0, "/opt/trn_rl_repo")

import numpy as np

H = W = C = 128
H2 = W2 = 256
NEG_SLOPE = 0.01


def _keys_cubic(t):
    t = np.abs(t)
    return np.where(
        t <= 1,
        (1.5 * t - 2.5) * t * t + 1,
        np.where(t < 2, ((-0.5 * t + 2.5) * t - 4) * t + 2, 0.0),
    )


def _resize_mat(n_in, n_out, antialias=True):
    """Replicates jax.image.resize(method='bicubic', antialias=True) weights.
    Returns (n_out, n_in) f32 so that y = Wmat @ x along the resized dim."""
    scale = n_out / n_in
    inv_scale = 1.0 / scale
    kernel_scale = max(inv_scale, 1.0) if antialias else 1.0
    sample_f = (np.arange(n_out, dtype=np.float64) + 0.5) * inv_scale - 0.5
    x = (
        np.abs(sample_f[:, None] - np.arange(n_in, dtype=np.float64)[None, :])
        / kernel_scale
    )
    w = _keys_cubic(x)
    total = w.sum(axis=1, keepdims=True)
    w = np.where(np.abs(total) > 1000 * np.finfo(np.float32).eps, w / total, 0)
    w = np.where(((sample_f >= -0.5) & (sample_f <= n_in - 0.5))[:, None], w, 0)
    return w.astype(np.float32)


_BUILD_CACHE = {}


def _build_module():
    """Build + compile the single-core Bass program (same program on all cores)."""
    if "nc" in _BUILD_CACHE:
        return _BUILD_CACHE["nc"]

    import concourse.bacc as bacc
    import concourse.mybir as mybir
    import concourse.tile as tile

    dt = mybir.dt

    nc = bacc.Bacc("TRN2", target_bir_lowering=False, debug=False)

    NQ = 8
    CQ = C // NQ
    xins = [
        nc.dram_tensor(f"xin{q}", (H, W * CQ), dt.float32r, kind="ExternalInput").ap()
        for q in range(NQ)
    ]
    wh = nc.dram_tensor("wh", (H, H2), dt.float32r, kind="ExternalInput").ap()
    ww = nc.dram_tensor("ww", (W, W2), dt.float32r, kind="ExternalInput").ap()
    dh = nc.dram_tensor("dh", (2, 128, 128), dt.bfloat16, kind="ExternalInput").ap()
    dw = nc.dram_tensor("dw", (2, 128, 128), dt.bfloat16, kind="ExternalInput").ap()
    youts = [
        nc.dram_tensor(f"yout{q}", (W, H * CQ), dt.float32, kind="ExternalOutput").ap()
        for q in range(NQ)
    ]

    AFT = mybir.ActivationFunctionType

    with tile.TileContext(nc) as tc:
        with (
            tc.tile_pool(name="big", bufs=1) as bigpool,
            tc.tile_pool(name="const", bufs=1) as cpool,
            tc.tile_pool(name="work", bufs=3) as wpool,
            tc.tile_pool(name="ps1", bufs=2, space="PSUM") as ps1,
            tc.tile_pool(name="ps2", bufs=2, space="PSUM") as ps2,
            tc.tile_pool(name="ps3", bufs=2, space="PSUM") as ps3,
        ):
            x_sbs = [bigpool.tile([H, W * CQ], dt.float32r, tag=f"x{q}", name=f"x_sb{q}") for q in range(NQ)]
            y_sbs = [bigpool.tile([W, H * CQ], dt.float32, tag=f"y{q}", name=f"y_sb{q}") for q in range(NQ)]
            wh_sb = cpool.tile([H, H2], dt.float32r)
            ww_sb = cpool.tile([W, W2], dt.float32r)
            dh_sb = cpool.tile([128, 256], dt.bfloat16)
            dw_sb = cpool.tile([128, 256], dt.bfloat16)

            x_rs = [t[:].rearrange("p (w c) -> p w c", c=CQ) for t in x_sbs]
            y_rs = [t[:].rearrange("p (w c) -> p w c", c=CQ) for t in y_sbs]
            nc.sync.dma_start(out=x_sbs[0][:], in_=xins[0][:])
            nc.sync.dma_start(out=wh_sb[:], in_=wh[:])
            nc.sync.dma_start(out=ww_sb[:], in_=ww[:])
            nc.sync.dma_start(out=dh_sb[:, 0:128], in_=dh[0])
            nc.sync.dma_start(out=dh_sb[:, 128:256], in_=dh[1])
            nc.sync.dma_start(out=dw_sb[:, 0:128], in_=dw[0])
            nc.sync.dma_start(out=dw_sb[:, 128:256], in_=dw[1])
            for q in range(1, NQ):
                nc.sync.dma_start(out=x_sbs[q][:], in_=xins[q][:])

            # Chain (contractions h, w, w2, h2; two axis-flips at Ph1/Ph3):
            #  Ph1 (flip, data-stationary): Z1t (w, h2) = X_c^T @ Uh
            #  Ph2 (standard, const stationary Ww halves): zT (w2half, h2)
            #  Ph3 (flip, data-stationary zsT): y3 (h2chunk, w3)
            #  Ph4 (standard, const stationary Dh halves): y (h3, w3)
            # Output partitions = h3 -> full-rate 64KB-row output DMA.
            # Channel pairs share PSUM tiles so evac instructions are 2x wider.
            sim_relu = os.environ.get("AF_SIM_RELU", "0") == "1"
            for cp in range(C // 2):
                c0 = 2 * cp
                # ---- Ph1: Z1t (w, h2) per channel ----
                p1t = ps1.tile([128, 512], dt.float32)
                for k in range(2):
                    nc.tensor.matmul(
                        p1t[:, k * 256 : k * 256 + 256],
                        lhsT=x_rs[(c0 + k) // CQ][:, :, (c0 + k) % CQ],
                        rhs=wh_sb[:],
                        start=True,
                        stop=True,
                    )
                z1 = wpool.tile([128, 512], dt.float32r, tag="z1")
                nc.vector.tensor_copy(out=z1[:], in_=p1t[:])

                # ---- Ph2: z (h2, w2) per channel; lhsT = Z1t halves ----
                p2t = ps2.tile([128, 1024], dt.float32)
                for k in range(2):
                    for b in range(2):
                        nc.tensor.matmul(
                            p2t[:, k * 512 + b * 256 : k * 512 + b * 256 + 256],
                            lhsT=z1[:, k * 256 + b * 128 : k * 256 + b * 128 + 128],
                            rhs=ww_sb[:],
                            start=True,
                            stop=True,
                        )
                # leaky_relu fused into the PSUM evacuation (ScalarE Prelu)
                zs = wpool.tile([128, 1024], dt.bfloat16, tag="zs")
                if sim_relu:
                    nc.scalar.activation(zs[:], p2t[:], AFT.Relu)
                else:
                    nc.scalar.activation(zs[:], p2t[:], AFT.Prelu, alpha=NEG_SLOPE)

                # ---- Ph3: y3 (w2half-a, h3) per channel; lhsT = zs slices ----
                p3t = ps3.tile([128, 512], dt.float32)
                for k in range(2):
                    for a in range(2):
                        o = k * 256 + a * 128
                        for b in range(2):
                            nc.tensor.matmul(
                                p3t[:, o : o + 128],
                                lhsT=zs[:, k * 512 + b * 256 + a * 128 : k * 512 + b * 256 + a * 128 + 128],
                                rhs=dh_sb[:, b * 128 : b * 128 + 128],
                                start=(b == 0),
                                stop=(b == 1),
                            )
                y3 = wpool.tile([128, 512], dt.bfloat16, tag="y3")
                if cp % 2 == 0:  # split e3 across ACT/DVE to balance engines
                    nc.scalar.activation(y3[:], p3t[:], AFT.Copy)
                else:
                    nc.vector.tensor_copy(out=y3[:], in_=p3t[:])

                # ---- Ph4: y (w3, h3), both channels per matmul (2D rhs) ----
                p4t = ps1.tile([128, 256], dt.float32, tag="p1t", padded_shape=[128, 512])
                y3_r = y3[:].rearrange("p (k a n) -> p k a n", k=2, a=2)
                p4_r = p4t[:].rearrange("p (k n) -> p k n", k=2)
                for a in range(2):
                    nc.tensor.matmul(
                        p4_r,
                        lhsT=dw_sb[:, a * 128 : a * 128 + 128],
                        rhs=y3_r[:, :, a, :],
                        start=(a == 0),
                        stop=(a == 1),
                    )
                nc.vector.tensor_copy(
                    out=y_rs[c0 // CQ][:, :, c0 % CQ : c0 % CQ + 2],
                    in_=p4t[:].rearrange("p (k n) -> p n k", k=2),
                )

            for q in range(NQ):
                nc.sync.dma_start(out=youts[q][:], in_=y_sbs[q][:])

    nc.compile()
    _BUILD_CACHE["nc"] = nc
    return nc


def _round_tf32(a):
    """Round f32 array to TF32 (10-bit mantissa, RNE) — what FP32R consumes."""
    v = np.ascontiguousarray(a, dtype=np.float32).view(np.uint32)
    lsb = (v >> np.uint32(13)) & np.uint32(1)
    v = v + np.uint32(0x0FFF) + lsb
    v = v & np.uint32(0xFFFFE000)
    return v.view(np.float32)


def _input_maps(x):
    U = _resize_mat(H, H2)   # (256, 128) upsample
    D = _resize_mat(H2, H)   # (128, 256) antialiased downsample
    try:
        from ml_dtypes import bfloat16
    except ImportError:
        import jax.numpy as jnp  # fallback
        bfloat16 = jnp.bfloat16

    wh_np = _round_tf32(np.ascontiguousarray(U.T))         # (h, h2) tf32
    ww_np = _round_tf32(np.ascontiguousarray(U.T))         # (w, w2) tf32
    # dh[b, h2local, h3] = D[h3, b*128 + h2local]
    dh_np = np.ascontiguousarray(D.T.reshape(2, 128, 128)).astype(bfloat16)
    dw_np = dh_np.copy()

    in_maps = []
    for i in range(x.shape[0]):
        xr = _round_tf32(x[i].reshape(H, W, C))
        m = {"wh": wh_np, "ww": ww_np, "dh": dh_np, "dw": dw_np}
        for q in range(8):
            m[f"xin{q}"] = np.ascontiguousarray(xr[:, :, q * 16 : (q + 1) * 16]).reshape(H, W * 16)
        in_maps.append(m)
    return in_maps


def _unshard(results):
    outs = []
    for r in results:
        qs = [np.asarray(r[f"yout{q}"]).reshape(W, H, 16) for q in range(8)]
        o = np.concatenate(qs, axis=2)              # (w3, h3, c)
        outs.append(np.transpose(o, (1, 0, 2)))
    return np.stack(outs, axis=0).astype(np.float32)


def run(x, trace=False):
    """Run on 8 NeuronCores. Returns (y, exec_time_ns or None)."""
    from concourse.bass_utils import run_bass_kernel_spmd

    nc = _build_module()
    in_maps = _input_maps(np.asarray(x, dtype=np.float32))
    core_ids = list(range(len(in_maps)))
    res = run_bass_kernel_spmd(nc, in_maps, core_ids, trace=trace)
    return _unshard(res.results), res.exec_time_ns


def kernel(x):
    y, _ = run(x, trace=False)
    return y


def _run_sim(x_batch):
    """CoreSim single-core numerical check (x_batch: (128,128,128) f32)."""
    import concourse.bass_interp as bass_interp

    nc = _build_module()
    sim = bass_interp.CoreSim(nc, trace=False)
    im = _input_maps(x_batch[None])[0]
    for k, v in im.items():
        sim.tensor(k)[:] = v
    sim.simulate()
    qs = [np.asarray(sim.tensor(f"yout{q}")).reshape(W, H, 16) for q in range(8)]
    o = np.concatenate(qs, axis=2)
    return np.transpose(o, (1, 0, 2))


# revision 25
# speedup vs baseline: 3.4941x; 3.4941x over previous
"""Trainium2 Bass kernel for nn_ActivationFilter:
y = bicubic_down2x( leaky_relu( bicubic_up2x(x) ) ), x: (8, 128, 128, 128) f32 NHWC.

Since jax.image.resize is a separable linear map, per (batch, channel):
    y = D @ leaky_relu(U @ X @ U^T) @ D^T
with U (256x128) the bicubic 2x-upsample matrix and D (128x256) the
antialiased bicubic downsample matrix.

Sharding: batch-per-core (8 batches over 8 NeuronCores), no collectives.

Per-core algorithm (per channel c, all on TensorE, no transposes):
  Ph1: Z1t[w, h2]  = sum_h  x[h, w, c] * Ut[h, h2]       (lhsT = X_c, f32r)
  Ph2: z[h2, w2]   = sum_w  Z1t[w, h2] * Ut[w, w2]       (lhsT = Z1t half, bf16)
       zs = leaky_relu(z)  (fused into PSUM evacuation on ScalarE)
  Ph3: y3[w2, h3]  = sum_h2 zs[h2, w2] * Dt[h2, h3]      (lhsT = zs slice, bf16)
  Ph4: y[w3, h3]   = sum_w2 Dt[w2, w3]^T-form @ y3       (lhsT = Dw, bf16)
Each phase's output partition dim is the next phase's contraction dim,
so no transposes are ever needed.
"""

import sys
import os

if "/opt/trn_rl_repo" not in sys.path:
    sys.path.insert(0, "/opt/trn_rl_repo")

import numpy as np

H = W = C = 128
H2 = W2 = 256
NEG_SLOPE = 0.01


def _keys_cubic(t):
    t = np.abs(t)
    return np.where(
        t <= 1,
        (1.5 * t - 2.5) * t * t + 1,
        np.where(t < 2, ((-0.5 * t + 2.5) * t - 4) * t + 2, 0.0),
    )


def _resize_mat(n_in, n_out, antialias=True):
    """Replicates jax.image.resize(method='bicubic', antialias=True) weights.
    Returns (n_out, n_in) f32 so that y = Wmat @ x along the resized dim."""
    scale = n_out / n_in
    inv_scale = 1.0 / scale
    kernel_scale = max(inv_scale, 1.0) if antialias else 1.0
    sample_f = (np.arange(n_out, dtype=np.float64) + 0.5) * inv_scale - 0.5
    x = (
        np.abs(sample_f[:, None] - np.arange(n_in, dtype=np.float64)[None, :])
        / kernel_scale
    )
    w = _keys_cubic(x)
    total = w.sum(axis=1, keepdims=True)
    w = np.where(np.abs(total) > 1000 * np.finfo(np.float32).eps, w / total, 0)
    w = np.where(((sample_f >= -0.5) & (sample_f <= n_in - 0.5))[:, None], w, 0)
    return w.astype(np.float32)


_BUILD_CACHE = {}


def _build_module():
    """Build + compile the single-core Bass program (same program on all cores)."""
    if "nc" in _BUILD_CACHE:
        return _BUILD_CACHE["nc"]

    import concourse.bacc as bacc
    import concourse.mybir as mybir
    import concourse.tile as tile

    dt = mybir.dt

    nc = bacc.Bacc("TRN2", target_bir_lowering=False, debug=False)

    NQ = 8
    CQ = C // NQ
    xins = [
        nc.dram_tensor(f"xin{q}", (H, W * CQ), dt.float32r, kind="ExternalInput").ap()
        for q in range(NQ)
    ]
    wh = nc.dram_tensor("wh", (H, H2), dt.float32r, kind="ExternalInput").ap()
    ww = nc.dram_tensor("ww", (W, W2), dt.float32r, kind="ExternalInput").ap()
    dh = nc.dram_tensor("dh", (2, 128, 128), dt.bfloat16, kind="ExternalInput").ap()
    dw = nc.dram_tensor("dw", (2, 128, 128), dt.bfloat16, kind="ExternalInput").ap()
    youts = [
        nc.dram_tensor(f"yout{q}", (W, H * CQ), dt.float32, kind="ExternalOutput").ap()
        for q in range(NQ)
    ]

    AFT = mybir.ActivationFunctionType

    with tile.TileContext(nc) as tc:
        with (
            tc.tile_pool(name="big", bufs=1) as bigpool,
            tc.tile_pool(name="const", bufs=1) as cpool,
            tc.tile_pool(name="work", bufs=6) as wpool,
            tc.tile_pool(name="ps1", bufs=2, space="PSUM") as ps1,
            tc.tile_pool(name="ps2", bufs=2, space="PSUM") as ps2,
            tc.tile_pool(name="ps3", bufs=1, space="PSUM") as ps3,
            tc.tile_pool(name="ps4", bufs=1, space="PSUM") as ps4,
        ):
            x_sbs = [bigpool.tile([H, W * CQ], dt.float32r, tag=f"x{q}", name=f"x_sb{q}") for q in range(NQ)]
            y_sbs = [bigpool.tile([W, H * CQ], dt.float32, tag=f"y{q}", name=f"y_sb{q}") for q in range(NQ)]
            wh_sb = cpool.tile([H, H2], dt.float32r)
            ww_sb = cpool.tile([W, W2], dt.float32r)
            dh_sb = cpool.tile([128, 256], dt.bfloat16)
            dw_sb = cpool.tile([128, 256], dt.bfloat16)

            x_rs = [t[:].rearrange("p (w c) -> p w c", c=CQ) for t in x_sbs]
            y_rs = [t[:].rearrange("p (w c) -> p w c", c=CQ) for t in y_sbs]
            nc.sync.dma_start(out=x_sbs[0][:], in_=xins[0][:])
            nc.sync.dma_start(out=wh_sb[:], in_=wh[:])
            nc.sync.dma_start(out=ww_sb[:], in_=ww[:])
            nc.sync.dma_start(out=dh_sb[:, 0:128], in_=dh[0])
            nc.sync.dma_start(out=dh_sb[:, 128:256], in_=dh[1])
            nc.sync.dma_start(out=dw_sb[:, 0:128], in_=dw[0])
            nc.sync.dma_start(out=dw_sb[:, 128:256], in_=dw[1])
            for q in range(1, NQ):
                nc.sync.dma_start(out=x_sbs[q][:], in_=xins[q][:])

            # Chain (contractions h, w, w2, h2; two axis-flips at Ph1/Ph3):
            #  Ph1 (flip, data-stationary): Z1t (w, h2) = X_c^T @ Uh
            #  Ph2 (standard, const stationary Ww halves): zT (w2half, h2)
            #  Ph3 (flip, data-stationary zsT): y3 (h2chunk, w3)
            #  Ph4 (standard, const stationary Dh halves): y (h3, w3)
            # Output partitions = h3 -> full-rate 64KB-row output DMA.
            # Channel pairs share PSUM tiles so evac instructions are 2x wider.
            sim_relu = os.environ.get("AF_SIM_RELU", "0") == "1"
            for cp in range(C // 2):
                c0 = 2 * cp
                # ---- Ph1: Z1t (w, h2) per channel ----
                p1t = ps1.tile([128, 512], dt.float32)
                for k in range(2):
                    nc.tensor.matmul(
                        p1t[:, k * 256 : k * 256 + 256],
                        lhsT=x_rs[(c0 + k) // CQ][:, :, (c0 + k) % CQ],
                        rhs=wh_sb[:],
                        start=True,
                        stop=True,
                    )
                z1 = wpool.tile([128, 512], dt.float32r, tag="z1")
                nc.vector.tensor_copy(out=z1[:], in_=p1t[:])

                # ---- Ph2: z (h2, w2) per channel; lhsT = Z1t halves ----
                p2t = ps2.tile([128, 1024], dt.float32)
                for k in range(2):
                    for b in range(2):
                        nc.tensor.matmul(
                            p2t[:, k * 512 + b * 256 : k * 512 + b * 256 + 256],
                            lhsT=z1[:, k * 256 + b * 128 : k * 256 + b * 128 + 128],
                            rhs=ww_sb[:],
                            start=True,
                            stop=True,
                        )
                # leaky_relu fused into the PSUM evacuation (ScalarE Prelu)
                zs = wpool.tile([128, 1024], dt.bfloat16, tag="zs")
                if sim_relu:
                    nc.scalar.activation(zs[:], p2t[:], AFT.Relu)
                else:
                    nc.scalar.activation(zs[:], p2t[:], AFT.Prelu, alpha=NEG_SLOPE)

                # ---- Ph3: y3 (w2half-a, h3) per channel; lhsT = zs slices ----
                p3t = ps3.tile([128, 512], dt.float32)
                for k in range(2):
                    for a in range(2):
                        o = k * 256 + a * 128
                        for b in range(2):
                            nc.tensor.matmul(
                                p3t[:, o : o + 128],
                                lhsT=zs[:, k * 512 + b * 256 + a * 128 : k * 512 + b * 256 + a * 128 + 128],
                                rhs=dh_sb[:, b * 128 : b * 128 + 128],
                                start=(b == 0),
                                stop=(b == 1),
                            )
                y3 = wpool.tile([128, 512], dt.bfloat16, tag="y3")
                if cp % 2 == 0:  # split e3 across ACT/DVE to balance engines
                    nc.scalar.activation(y3[:], p3t[:], AFT.Copy)
                else:
                    nc.vector.tensor_copy(out=y3[:], in_=p3t[:])

                # ---- Ph4: y (w3, h3), both channels per matmul (2D rhs) ----
                p4t = ps4.tile([128, 256], dt.float32)
                y3_r = y3[:].rearrange("p (k a n) -> p k a n", k=2, a=2)
                p4_r = p4t[:].rearrange("p (k n) -> p k n", k=2)
                for a in range(2):
                    nc.tensor.matmul(
                        p4_r,
                        lhsT=dw_sb[:, a * 128 : a * 128 + 128],
                        rhs=y3_r[:, :, a, :],
                        start=(a == 0),
                        stop=(a == 1),
                    )
                nc.vector.tensor_copy(
                    out=y_rs[c0 // CQ][:, :, c0 % CQ : c0 % CQ + 2],
                    in_=p4t[:].rearrange("p (k n) -> p n k", k=2),
                )

            for q in range(NQ):
                nc.sync.dma_start(out=youts[q][:], in_=y_sbs[q][:])

    nc.compile()
    _BUILD_CACHE["nc"] = nc
    return nc


def _round_tf32(a):
    """Round f32 array to TF32 (10-bit mantissa, RNE) — what FP32R consumes."""
    v = np.ascontiguousarray(a, dtype=np.float32).view(np.uint32)
    lsb = (v >> np.uint32(13)) & np.uint32(1)
    v = v + np.uint32(0x0FFF) + lsb
    v = v & np.uint32(0xFFFFE000)
    return v.view(np.float32)


def _input_maps(x):
    U = _resize_mat(H, H2)   # (256, 128) upsample
    D = _resize_mat(H2, H)   # (128, 256) antialiased downsample
    try:
        from ml_dtypes import bfloat16
    except ImportError:
        import jax.numpy as jnp  # fallback
        bfloat16 = jnp.bfloat16

    wh_np = _round_tf32(np.ascontiguousarray(U.T))         # (h, h2) tf32
    ww_np = _round_tf32(np.ascontiguousarray(U.T))         # (w, w2) tf32
    # dh[b, h2local, h3] = D[h3, b*128 + h2local]
    dh_np = np.ascontiguousarray(D.T.reshape(2, 128, 128)).astype(bfloat16)
    dw_np = dh_np.copy()

    in_maps = []
    for i in range(x.shape[0]):
        xr = _round_tf32(x[i].reshape(H, W, C))
        m = {"wh": wh_np, "ww": ww_np, "dh": dh_np, "dw": dw_np}
        for q in range(8):
            m[f"xin{q}"] = np.ascontiguousarray(xr[:, :, q * 16 : (q + 1) * 16]).reshape(H, W * 16)
        in_maps.append(m)
    return in_maps


def _unshard(results):
    outs = []
    for r in results:
        qs = [np.asarray(r[f"yout{q}"]).reshape(W, H, 16) for q in range(8)]
        o = np.concatenate(qs, axis=2)              # (w3, h3, c)
        outs.append(np.transpose(o, (1, 0, 2)))
    return np.stack(outs, axis=0).astype(np.float32)


def run(x, trace=False):
    """Run on 8 NeuronCores. Returns (y, exec_time_ns or None)."""
    from concourse.bass_utils import run_bass_kernel_spmd

    nc = _build_module()
    in_maps = _input_maps(np.asarray(x, dtype=np.float32))
    core_ids = list(range(len(in_maps)))
    res = run_bass_kernel_spmd(nc, in_maps, core_ids, trace=trace)
    return _unshard(res.results), res.exec_time_ns


def kernel(x):
    y, _ = run(x, trace=False)
    return y


def _run_sim(x_batch):
    """CoreSim single-core numerical check (x_batch: (128,128,128) f32)."""
    import concourse.bass_interp as bass_interp

    nc = _build_module()
    sim = bass_interp.CoreSim(nc, trace=False)
    im = _input_maps(x_batch[None])[0]
    for k, v in im.items():
        sim.tensor(k)[:] = v
    sim.simulate()
    qs = [np.asarray(sim.tensor(f"yout{q}")).reshape(W, H, 16) for q in range(8)]
    o = np.concatenate(qs, axis=2)
    return np.transpose(o, (1, 0, 2))


# revision 26
# speedup vs baseline: 3.5820x; 1.0252x over previous
"""Trainium2 Bass kernel for nn_ActivationFilter:
y = bicubic_down2x( leaky_relu( bicubic_up2x(x) ) ), x: (8, 128, 128, 128) f32 NHWC.

Since jax.image.resize is a separable linear map, per (batch, channel):
    y = D @ leaky_relu(U @ X @ U^T) @ D^T
with U (256x128) the bicubic 2x-upsample matrix and D (128x256) the
antialiased bicubic downsample matrix.

Sharding: batch-per-core (8 batches over 8 NeuronCores), no collectives.

Per-core algorithm (per channel c, all on TensorE, no transposes):
  Ph1: Z1t[w, h2]  = sum_h  x[h, w, c] * Ut[h, h2]       (lhsT = X_c, f32r)
  Ph2: z[h2, w2]   = sum_w  Z1t[w, h2] * Ut[w, w2]       (lhsT = Z1t half, bf16)
       zs = leaky_relu(z)  (fused into PSUM evacuation on ScalarE)
  Ph3: y3[w2, h3]  = sum_h2 zs[h2, w2] * Dt[h2, h3]      (lhsT = zs slice, bf16)
  Ph4: y[w3, h3]   = sum_w2 Dt[w2, w3]^T-form @ y3       (lhsT = Dw, bf16)
Each phase's output partition dim is the next phase's contraction dim,
so no transposes are ever needed.
"""

import sys
import os

if "/opt/trn_rl_repo" not in sys.path:
    sys.path.insert(0, "/opt/trn_rl_repo")

import numpy as np

H = W = C = 128
H2 = W2 = 256
NEG_SLOPE = 0.01


def _keys_cubic(t):
    t = np.abs(t)
    return np.where(
        t <= 1,
        (1.5 * t - 2.5) * t * t + 1,
        np.where(t < 2, ((-0.5 * t + 2.5) * t - 4) * t + 2, 0.0),
    )


def _resize_mat(n_in, n_out, antialias=True):
    """Replicates jax.image.resize(method='bicubic', antialias=True) weights.
    Returns (n_out, n_in) f32 so that y = Wmat @ x along the resized dim."""
    scale = n_out / n_in
    inv_scale = 1.0 / scale
    kernel_scale = max(inv_scale, 1.0) if antialias else 1.0
    sample_f = (np.arange(n_out, dtype=np.float64) + 0.5) * inv_scale - 0.5
    x = (
        np.abs(sample_f[:, None] - np.arange(n_in, dtype=np.float64)[None, :])
        / kernel_scale
    )
    w = _keys_cubic(x)
    total = w.sum(axis=1, keepdims=True)
    w = np.where(np.abs(total) > 1000 * np.finfo(np.float32).eps, w / total, 0)
    w = np.where(((sample_f >= -0.5) & (sample_f <= n_in - 0.5))[:, None], w, 0)
    return w.astype(np.float32)


_BUILD_CACHE = {}


def _build_module():
    """Build + compile the single-core Bass program (same program on all cores)."""
    if "nc" in _BUILD_CACHE:
        return _BUILD_CACHE["nc"]

    import concourse.bacc as bacc
    import concourse.mybir as mybir
    import concourse.tile as tile

    dt = mybir.dt

    nc = bacc.Bacc("TRN2", target_bir_lowering=False, debug=False)

    NQ = 16
    CQ = C // NQ
    xins = [
        nc.dram_tensor(f"xin{q}", (H, W * CQ), dt.float32r, kind="ExternalInput").ap()
        for q in range(NQ)
    ]
    wh = nc.dram_tensor("wh", (H, H2), dt.float32r, kind="ExternalInput").ap()
    ww = nc.dram_tensor("ww", (W, W2), dt.float32r, kind="ExternalInput").ap()
    dh = nc.dram_tensor("dh", (2, 128, 128), dt.bfloat16, kind="ExternalInput").ap()
    dw = nc.dram_tensor("dw", (2, 128, 128), dt.bfloat16, kind="ExternalInput").ap()
    youts = [
        nc.dram_tensor(f"yout{q}", (W, H * CQ), dt.float32, kind="ExternalOutput").ap()
        for q in range(NQ)
    ]

    AFT = mybir.ActivationFunctionType

    with tile.TileContext(nc) as tc:
        with (
            tc.tile_pool(name="big", bufs=1) as bigpool,
            tc.tile_pool(name="const", bufs=1) as cpool,
            tc.tile_pool(name="work", bufs=6) as wpool,
            tc.tile_pool(name="ps1", bufs=2, space="PSUM") as ps1,
            tc.tile_pool(name="ps2", bufs=2, space="PSUM") as ps2,
            tc.tile_pool(name="ps3", bufs=1, space="PSUM") as ps3,
            tc.tile_pool(name="ps4", bufs=1, space="PSUM") as ps4,
        ):
            x_sbs = [bigpool.tile([H, W * CQ], dt.float32r, tag=f"x{q}", name=f"x_sb{q}") for q in range(NQ)]
            y_sbs = [bigpool.tile([W, H * CQ], dt.float32, tag=f"y{q}", name=f"y_sb{q}") for q in range(NQ)]
            wh_sb = cpool.tile([H, H2], dt.float32r)
            ww_sb = cpool.tile([W, W2], dt.float32r)
            dh_sb = cpool.tile([128, 256], dt.bfloat16)
            dw_sb = cpool.tile([128, 256], dt.bfloat16)

            x_rs = [t[:].rearrange("p (w c) -> p w c", c=CQ) for t in x_sbs]
            y_rs = [t[:].rearrange("p (w c) -> p w c", c=CQ) for t in y_sbs]
            nc.sync.dma_start(out=x_sbs[0][:], in_=xins[0][:])
            nc.sync.dma_start(out=wh_sb[:], in_=wh[:])
            nc.sync.dma_start(out=ww_sb[:], in_=ww[:])
            nc.sync.dma_start(out=dh_sb[:, 0:128], in_=dh[0])
            nc.sync.dma_start(out=dh_sb[:, 128:256], in_=dh[1])
            nc.sync.dma_start(out=dw_sb[:, 0:128], in_=dw[0])
            nc.sync.dma_start(out=dw_sb[:, 128:256], in_=dw[1])
            for q in range(1, NQ):
                nc.sync.dma_start(out=x_sbs[q][:], in_=xins[q][:])

            # Chain (contractions h, w, w2, h2; two axis-flips at Ph1/Ph3):
            #  Ph1 (flip, data-stationary): Z1t (w, h2) = X_c^T @ Uh
            #  Ph2 (standard, const stationary Ww halves): zT (w2half, h2)
            #  Ph3 (flip, data-stationary zsT): y3 (h2chunk, w3)
            #  Ph4 (standard, const stationary Dh halves): y (h3, w3)
            # Output partitions = h3 -> full-rate 64KB-row output DMA.
            # Channel pairs share PSUM tiles so evac instructions are 2x wider.
            sim_relu = os.environ.get("AF_SIM_RELU", "0") == "1"
            for cp in range(C // 2):
                c0 = 2 * cp
                # ---- Ph1: Z1t (w, h2) per channel ----
                p1t = ps1.tile([128, 512], dt.float32)
                for k in range(2):
                    nc.tensor.matmul(
                        p1t[:, k * 256 : k * 256 + 256],
                        lhsT=x_rs[(c0 + k) // CQ][:, :, (c0 + k) % CQ],
                        rhs=wh_sb[:],
                        start=True,
                        stop=True,
                    )
                z1 = wpool.tile([128, 512], dt.float32r, tag="z1")
                nc.vector.tensor_copy(out=z1[:], in_=p1t[:])

                # ---- Ph2: z (h2, w2) per channel; lhsT = Z1t halves ----
                p2t = ps2.tile([128, 1024], dt.float32)
                for k in range(2):
                    for b in range(2):
                        nc.tensor.matmul(
                            p2t[:, k * 512 + b * 256 : k * 512 + b * 256 + 256],
                            lhsT=z1[:, k * 256 + b * 128 : k * 256 + b * 128 + 128],
                            rhs=ww_sb[:],
                            start=True,
                            stop=True,
                        )
                # leaky_relu fused into the PSUM evacuation (ScalarE Prelu)
                zs = wpool.tile([128, 1024], dt.bfloat16, tag="zs")
                if sim_relu:
                    nc.scalar.activation(zs[:], p2t[:], AFT.Relu)
                else:
                    nc.scalar.activation(zs[:], p2t[:], AFT.Prelu, alpha=NEG_SLOPE)

                # ---- Ph3: y3 (w2half-a, h3) per channel; lhsT = zs slices ----
                p3t = ps3.tile([128, 512], dt.float32)
                for k in range(2):
                    for a in range(2):
                        o = k * 256 + a * 128
                        for b in range(2):
                            nc.tensor.matmul(
                                p3t[:, o : o + 128],
                                lhsT=zs[:, k * 512 + b * 256 + a * 128 : k * 512 + b * 256 + a * 128 + 128],
                                rhs=dh_sb[:, b * 128 : b * 128 + 128],
                                start=(b == 0),
                                stop=(b == 1),
                            )
                y3 = wpool.tile([128, 512], dt.bfloat16, tag="y3")
                if cp % 2 == 0:  # split e3 across ACT/DVE to balance engines
                    nc.scalar.activation(y3[:], p3t[:], AFT.Copy)
                else:
                    nc.vector.tensor_copy(out=y3[:], in_=p3t[:])

                # ---- Ph4: y (w3, h3), both channels per matmul (2D rhs) ----
                p4t = ps4.tile([128, 256], dt.float32)
                y3_r = y3[:].rearrange("p (k a n) -> p k a n", k=2, a=2)
                p4_r = p4t[:].rearrange("p (k n) -> p k n", k=2)
                for a in range(2):
                    nc.tensor.matmul(
                        p4_r,
                        lhsT=dw_sb[:, a * 128 : a * 128 + 128],
                        rhs=y3_r[:, :, a, :],
                        start=(a == 0),
                        stop=(a == 1),
                    )
                nc.vector.tensor_copy(
                    out=y_rs[c0 // CQ][:, :, c0 % CQ : c0 % CQ + 2],
                    in_=p4t[:].rearrange("p (k n) -> p n k", k=2),
                )

            for q in range(NQ):
                nc.sync.dma_start(out=youts[q][:], in_=y_sbs[q][:])

    nc.compile()
    _BUILD_CACHE["nc"] = nc
    return nc


def _round_tf32(a):
    """Round f32 array to TF32 (10-bit mantissa, RNE) — what FP32R consumes."""
    v = np.ascontiguousarray(a, dtype=np.float32).view(np.uint32)
    lsb = (v >> np.uint32(13)) & np.uint32(1)
    v = v + np.uint32(0x0FFF) + lsb
    v = v & np.uint32(0xFFFFE000)
    return v.view(np.float32)


def _input_maps(x):
    U = _resize_mat(H, H2)   # (256, 128) upsample
    D = _resize_mat(H2, H)   # (128, 256) antialiased downsample
    try:
        from ml_dtypes import bfloat16
    except ImportError:
        import jax.numpy as jnp  # fallback
        bfloat16 = jnp.bfloat16

    wh_np = _round_tf32(np.ascontiguousarray(U.T))         # (h, h2) tf32
    ww_np = _round_tf32(np.ascontiguousarray(U.T))         # (w, w2) tf32
    # dh[b, h2local, h3] = D[h3, b*128 + h2local]
    dh_np = np.ascontiguousarray(D.T.reshape(2, 128, 128)).astype(bfloat16)
    dw_np = dh_np.copy()

    in_maps = []
    for i in range(x.shape[0]):
        xr = _round_tf32(x[i].reshape(H, W, C))
        m = {"wh": wh_np, "ww": ww_np, "dh": dh_np, "dw": dw_np}
        for q in range(16):
            m[f"xin{q}"] = np.ascontiguousarray(xr[:, :, q * 8 : (q + 1) * 8]).reshape(H, W * 8)
        in_maps.append(m)
    return in_maps


def _unshard(results):
    outs = []
    for r in results:
        qs = [np.asarray(r[f"yout{q}"]).reshape(W, H, 8) for q in range(16)]
        o = np.concatenate(qs, axis=2)              # (w3, h3, c)
        outs.append(np.transpose(o, (1, 0, 2)))
    return np.stack(outs, axis=0).astype(np.float32)


def run(x, trace=False):
    """Run on 8 NeuronCores. Returns (y, exec_time_ns or None)."""
    from concourse.bass_utils import run_bass_kernel_spmd

    nc = _build_module()
    in_maps = _input_maps(np.asarray(x, dtype=np.float32))
    core_ids = list(range(len(in_maps)))
    res = run_bass_kernel_spmd(nc, in_maps, core_ids, trace=trace)
    return _unshard(res.results), res.exec_time_ns


def kernel(x):
    y, _ = run(x, trace=False)
    return y


def _run_sim(x_batch):
    """CoreSim single-core numerical check (x_batch: (128,128,128) f32)."""
    import concourse.bass_interp as bass_interp

    nc = _build_module()
    sim = bass_interp.CoreSim(nc, trace=False)
    im = _input_maps(x_batch[None])[0]
    for k, v in im.items():
        sim.tensor(k)[:] = v
    sim.simulate()
    qs = [np.asarray(sim.tensor(f"yout{q}")).reshape(W, H, 8) for q in range(16)]
    o = np.concatenate(qs, axis=2)
    return np.transpose(o, (1, 0, 2))
